# revision 3
# baseline (speedup 1.0000x reference)
"""Trainium2 Bass kernel for nn_DiffLogicPBF (difflogic network).

Algorithm
---------
The network input is binarized to 2 bits, so every batch row's activation
trajectory takes one of only 4 "patterns".  We evaluate the network on the 4
patterns instead of 8192 rows, then blend per-row.

The per-layer gathers are composed on the host into a stream tree (layer l is
evaluated 2^(5-l) times in permuted orders, 63 streams total), so the device
does gather-free elementwise work only.  Weights arrive pre-permuted, in fp8.

Device pipeline per core (512 neurons x 63 streams = 252 columns of 128):
  exp(w) on ACT  ->  PE matmul with a constant 16->5 matrix that computes the
  multilinear coefficients c0..c3 and the softmax denominator D per neuron
  ->  r = 1/D per column, then one fused PSUM->SBUF multiply that both
  converts and NORMALIZES the coefficients (slab = psum * r)  ->
  elementwise multilinear evaluation over the 4 patterns; the pattern dim is
  split across engines (DVE takes patterns 0-2, GpSimd pattern 3) so the two
  engines evaluate the serial layer chain in parallel.

Tail: the 4-entry logit table is summed over partitions by a ones-matmul on
H5 directly, folded, differenced, and blended per batch row with 3 fused ops.
Each core emits one [128,64] plane (its class partial); the host assembles.

Sharding: neurons are split across the 8 cores (512 each); cores 0-3 are
class-0 partials, 4-7 class-1.
"""

from contextlib import ExitStack

import ml_dtypes
import numpy as np

import concourse.bacc as bacc
import concourse.mybir as mybir
import concourse.tile as tile
from concourse.bass_utils import run_bass_kernel_spmd

F32 = mybir.dt.float32
BF16 = mybir.dt.bfloat16
FP8 = mybir.dt.float8e4
ADD = mybir.AluOpType.add
SUB = mybir.AluOpType.subtract
MUL = mybir.AluOpType.mult
GT = mybir.AluOpType.is_gt
X = mybir.AxisListType.X
EXP = mybir.ActivationFunctionType.Exp

N_CORES = 8
B, K, L = 8192, 4096, 6
NS = [32, 16, 8, 4, 2, 1]            # streams per layer
FO = np.cumsum([0] + NS).tolist()    # stream offsets by layer
COLB = [f * 4 for f in FO]           # column base per layer
BROW = B // 128

GSPLIT = True                        # GpSimd evaluates pattern 3

_compiled = None


def _build_program():
    nc = bacc.Bacc("TRN2", target_bir_lowering=False, debug=False,
                   num_devices=N_CORES)
    wallin = nc.dram_tensor("wallin", [128, 4096], FP8, kind="ExternalInput").ap()
    abxkin = nc.dram_tensor("abxkin", [128, 1192], BF16, kind="ExternalInput").ap()
    out = nc.dram_tensor("out", [128, BROW], F32, kind="ExternalOutput").ap()

    with tile.TileContext(nc) as tc:
        with ExitStack() as ctx:
            p = ctx.enter_context(tc.tile_pool(name="p", bufs=1))
            psp = ctx.enter_context(tc.tile_pool(name="ps", bufs=1, space="PSUM"))

            # ---- input DMAs first: completion latency overlaps setup ----
            wall = p.tile([128, 4096], FP8)
            abxk = p.tile([128, 1192], BF16)
            nc.sync.dma_start(wall[:, 0:1024], wallin[:, 0:1024])
            nc.scalar.dma_start(wall[:, 2048:4096], wallin[:, 2048:4096])
            nc.sync.dma_start(wall[:, 1024:2048], wallin[:, 1024:2048])
            nc.gpsimd.dma_start(abxk[:], abxkin[:])

            av = abxk[:, 0:512].rearrange("p (q c) -> p q c", c=128)
            bv = abxk[:, 512:1024].rearrange("p (q c) -> p q c", c=128)
            xv = abxk[:, 1024:1152].rearrange("p (a c) -> p a c", c=2)
            kmv = abxk[:, 1152:1192]

            # ones for the table-broadcast matmul, built on device
            onesb = p.tile([128, 128], BF16)
            nc.gpsimd.memset(onesb[:], 1.0)

            # blend prep on DVE while weights are in flight
            s0 = p.tile([128, BROW], F32)
            nc.vector.tensor_scalar(s0[:], xv[:, :, 0], 0.0, None, op0=GT)
            s1 = p.tile([128, BROW], F32)
            nc.vector.tensor_scalar(s1[:], xv[:, :, 1], 0.0, None, op0=GT)
            t01 = p.tile([128, BROW], F32)
            nc.vector.tensor_tensor(t01[:], s0[:], s1[:], op=MUL)

            # ---- exp on ACT, 4 chunks ----
            E = p.tile([128, 4096], BF16)
            for k in range(4):
                nc.scalar.activation(E[:, k * 1024:(k + 1) * 1024],
                                     wall[:, k * 1024:(k + 1) * 1024], EXP)

            # ---- coefficient matmuls: one per 8-column chunk ----
            psb = [psp.tile([128, 320], F32, tag=f"pb{b}", name=f"pb{b}")
                   for b in range(4)]
            for c in range(32):
                b, s = c // 8, c % 8
                nc.tensor.matmul(psb[b][:, s * 40:(s + 1) * 40],
                                 E[:, c * 128:(c + 1) * 128], kmv,
                                 start=True, stop=True)

            # ---- per-bank: r = 1/D, then normalize+convert coefficients ----
            slabK = p.tile([128, 4, 256], BF16)
            rall = p.tile([128, 256], F32)

            def recip_bank(b):
                dflat = psb[b][:].rearrange("p (sg k) -> p sg k", k=5)
                rv = rall[:, b * 64:(b + 1) * 64]
                nc.vector.reciprocal_approx_fast(rv, dflat[:, :, 4])

            def norm_bank(b):
                pv = psb[b][:].rearrange("p (s g k) -> p k s g", k=5, g=8)
                outv = slabK[:, :, b * 64:(b + 1) * 64].rearrange(
                    "p k (s g) -> p k s g", g=8)
                rb = rall[:, b * 64:(b + 1) * 64].rearrange(
                    "p (s g) -> p s g", g=8).unsqueeze(1).broadcast_to(
                    [128, 4, 8, 8])
                nc.vector.tensor_tensor(outv, pv[:, 0:4], rb, op=MUL)

            H = {}
            for l in range(L):
                n = COLB[l + 1] - COLB[l]
                H[l] = p.tile([128, 4, n], BF16, tag=f"H{l}", name=f"H{l}")

            def eval_piece(l, lo, hi, tag, eng, q0, q1):
                # V = [t | u] double-wide: t = B*c3 + c1, u = B*c2 + c0,
                # then H = t*A + u.  Patterns [q0:q1) only.
                n = hi - lo
                nq = q1 - q0
                if l == 0:
                    A = av[:, q0:q1, lo:hi]
                    BB = bv[:, q0:q1, lo:hi].unsqueeze(2).broadcast_to(
                        [128, nq, 2, n])
                else:
                    Hp = H[l - 1]
                    A = Hp[:, q0:q1, 0:n]
                    BB = Hp[:, q0:q1, n:2 * n].unsqueeze(2).broadcast_to(
                        [128, nq, 2, n])
                c32 = slabK[:, 0:2, lo:hi].unsqueeze(1).broadcast_to(
                    [128, nq, 2, n])
                c10 = slabK[:, 2:4, lo:hi].unsqueeze(1).broadcast_to(
                    [128, nq, 2, n])
                llo = lo - COLB[l]
                Hv = H[l][:, q0:q1, llo:llo + n]
                V = p.tile([128, nq, 2, n], BF16, tag=f"V{tag}", name=f"V{tag}")
                eng.tensor_tensor(V[:], BB, c32, op=MUL)
                eng.tensor_tensor(V[:], V[:], c10, op=ADD)
                m = p.tile([128, nq, n], BF16, tag=f"m{tag}", name=f"m{tag}")
                eng.tensor_tensor(m[:], V[:, :, 0], A, op=MUL)
                eng.tensor_tensor(Hv, m[:], V[:, :, 1], op=ADD)

            PIECES = [(0, 0, 64, "l0a"), (0, 64, 128, "l0b"),
                      (1, 128, 192, "l1"), (2, 192, 224, "l2"),
                      (3, 224, 240, "l3"), (4, 240, 248, "l4"),
                      (5, 248, 252, "l5")]

            def vpiece(i):
                l, lo, hi, tag = PIECES[i]
                if GSPLIT:
                    eval_piece(l, lo, hi, tag + "v", nc.vector, 0, 3)
                else:
                    eval_piece(l, lo, hi, tag + "v", nc.vector, 0, 4)

            # DVE emission: coeff for bank b, then the eval piece it gates
            recip_bank(0)
            norm_bank(0)
            vpiece(0)
            recip_bank(1)
            norm_bank(1)
            vpiece(1)
            recip_bank(2)
            norm_bank(2)
            vpiece(2)
            recip_bank(3)
            norm_bank(3)
            for i in range(3, 7):
                vpiece(i)

            # GpSimd runs pattern 3 of every piece in parallel
            if GSPLIT:
                for i in range(7):
                    l, lo, hi, tag = PIECES[i]
                    eval_piece(l, lo, hi, tag + "g", nc.gpsimd, 3, 4)

            # ---- partial GroupSum: ones-matmul sums H5 over partitions ----
            psg = psp.tile([128, 16], F32)
            nc.tensor.matmul(psg[:], onesb[:],
                             H[5][:].rearrange("p q c -> p (q c)"),
                             start=True, stop=True)

            pr = psg[:].rearrange("p (q c) -> p q c", c=4)
            T = p.tile([128, 4], F32)
            nc.vector.tensor_reduce(T[:], pr[:], axis=X, op=ADD)
            # U = [d10, d01, T11-T00];  d11 = U2 - U0 - U1
            U = p.tile([128, 3], F32)
            nc.vector.tensor_tensor(U[:], T[:, 1:4],
                                    T[:, 0:1].broadcast_to([128, 3]), op=SUB)
            dt = p.tile([128, 1], F32)
            nc.vector.scalar_tensor_tensor(dt[:], U[:, 2:3], U[:, 0:1],
                                           U[:, 1:2], op0=SUB, op1=SUB)

            # ---- per-row blend of the full batch (scalars from tail tiles) ----
            ev = p.tile([128, BROW], F32)
            nc.vector.tensor_scalar(ev[:], s0[:], U[:, 0:1], T[:, 0:1],
                                    op0=MUL, op1=ADD)
            z1 = p.tile([128, BROW], F32)
            nc.vector.scalar_tensor_tensor(z1[:], s1[:], U[:, 1:2], ev[:],
                                           op0=MUL, op1=ADD)
            zo = p.tile([128, BROW], F32)
            nc.vector.scalar_tensor_tensor(zo[:], t01[:], dt[:, 0:1], z1[:],
                                           op0=MUL, op1=ADD)
            nc.sync.dma_start(out, zo[:])

    nc.compile()
    return nc


def _host_blobs(x, w0, ws, idx0, idxs):
    """Compose the stream tree and build per-core input blobs."""
    x = np.asarray(x, np.float32)
    Wl = [np.asarray(w0, np.float32)] + [np.asarray(ws[i], np.float32)
                                         for i in range(L - 1)]
    Il = [np.asarray(idx0, np.int64)] + [np.asarray(idxs[i], np.int64)
                                         for i in range(L - 1)]

    S = [None] * L
    S[L - 1] = [np.arange(K)]
    for l in range(L - 1, 0, -1):
        S[l - 1] = [Il[l][0][P] for P in S[l]] + [Il[l][1][P] for P in S[l]]

    # weights in column order: wtmp[core, col, p, i], col = 4*stream + j
    wtmp = np.zeros((N_CORES, 256, 128, 16), np.float32)
    for l in range(L):
        for s in range(NS[l]):
            sg = FO[l] + s
            pw = Wl[l][S[l][s]].reshape(N_CORES, 4, 128, 16)
            wtmp[:, sg * 4:(sg + 1) * 4] = pw
    # wall[core, g*16+i, c*128+p], col = c*8+g
    wt = wtmp.reshape(N_CORES, 32, 8, 128, 16)
    wall = np.ascontiguousarray(
        wt.transpose(0, 2, 4, 1, 3).reshape(N_CORES, 128, 4096))

    # layer-0 pattern inputs, pattern-major: a0[core, p, q*128 + col]
    q = np.arange(4)
    msel0 = np.zeros((N_CORES, 128, 128), np.int64)  # [core, col, p]
    msel1 = np.zeros((N_CORES, 128, 128), np.int64)
    for s in range(NS[0]):
        idx = S[0][s].reshape(N_CORES, 4, 128)
        msel0[:, s * 4:(s + 1) * 4] = Il[0][0][idx]
        msel1[:, s * 4:(s + 1) * 4] = Il[0][1][idx]
    a0 = (q[None, :, None, None] >> msel0[:, None, :, :]) & 1   # [core,q,col,p]
    b0 = (q[None, :, None, None] >> msel1[:, None, :, :]) & 1
    a0 = a0.transpose(0, 3, 1, 2).reshape(N_CORES, 128, 512)    # [core,p,(q,col)]
    b0 = b0.transpose(0, 3, 1, 2).reshape(N_CORES, 128, 512)

    # constant 16->5 coefficient matrix, block-diagonal over 8 groups
    i16 = np.arange(16)
    t11, t10 = i16 & 1, (i16 >> 1) & 1
    t01, t00 = (i16 >> 2) & 1, (i16 >> 3) & 1
    KC = np.stack([t11 - t10 - t01 + t00, t01 - t00, t10 - t00,
                   t00, np.ones(16, np.int64)], 1)  # [16,5]: c3,c2,c1,c0,D
    kb = np.zeros((8, 16, 8, 5), np.float32)
    for gidx in range(8):
        kb[gidx, :, gidx, :] = KC
    kblob = kb.reshape(128, 40)

    xpart = np.ascontiguousarray(x.reshape(128, 128))

    in_maps = []
    for ci in range(N_CORES):
        abm = np.concatenate([a0[ci], b0[ci], xpart, kblob], axis=1)
        in_maps.append({
            "wallin": wall[ci].astype(ml_dtypes.float8_e4m3fn),
            "abxkin": np.ascontiguousarray(abm).astype(ml_dtypes.bfloat16),
        })
    return in_maps


def run(inputs, trace=False, trace_kwargs=None):
    global _compiled
    if _compiled is None:
        _compiled = _build_program()
    nc = _compiled
    in_maps = _host_blobs(inputs["x"], inputs["w0"], inputs["ws"],
                          inputs["idx0"], inputs["idxs"])
    res = run_bass_kernel_spmd(nc, in_maps, core_ids=list(range(N_CORES)),
                               trace=trace, **(trace_kwargs or {}))
    total = np.zeros((B, 2), np.float32)
    for ci in range(N_CORES):
        total[:, 0 if ci < N_CORES // 2 else 1] += \
            np.asarray(res.results[ci]["out"], np.float32).reshape(B)
    return total, res


def kernel(x, w0, ws, idx0, idxs):
    out, _ = run({"x": x, "w0": w0, "ws": ws, "idx0": idx0, "idxs": idxs})
    return out


# revision 9
# speedup vs baseline: 1.0814x; 1.0814x over previous
"""Trainium2 Bass kernel for nn_DiffLogicPBF (difflogic network).

Algorithm
---------
The network input is binarized to 2 bits, so every batch row's activation
trajectory takes one of only 4 "patterns".  We evaluate the network on the 4
patterns instead of 8192 rows, then blend per-row.

The per-layer gathers are composed on the host into a stream tree (layer l is
evaluated 2^(5-l) times in permuted orders, 63 streams total), so the device
does gather-free elementwise work only.  Weights arrive pre-permuted, in fp8.

Device pipeline per core (512 neurons x 63 streams = 252 columns of 128):
  exp(w) on ACT  ->  PE matmul with a constant 16->5 matrix that computes the
  multilinear coefficients c0..c3 and the softmax denominator D per neuron
  ->  r = 1/D per column, then one fused PSUM->SBUF multiply that both
  converts and NORMALIZES the coefficients (slab = psum * r)  ->
  elementwise multilinear evaluation over the 4 patterns; the pattern dim is
  split across engines (DVE takes patterns 0-2, GpSimd pattern 3) so the two
  engines evaluate the serial layer chain in parallel.

Tail: the 4-entry logit table is summed over partitions by a ones-matmul on
H5 directly, folded, differenced, and blended per batch row with 3 fused ops.
Each core emits one [128,64] plane (its class partial); the host assembles.

Sharding: neurons are split across the 8 cores (512 each); cores 0-3 are
class-0 partials, 4-7 class-1.
"""

from contextlib import ExitStack

import ml_dtypes
import numpy as np

import concourse.bacc as bacc
import concourse.mybir as mybir
import concourse.tile as tile
from concourse.bass_utils import run_bass_kernel_spmd

F32 = mybir.dt.float32
BF16 = mybir.dt.bfloat16
FP8 = mybir.dt.float8e4
ADD = mybir.AluOpType.add
SUB = mybir.AluOpType.subtract
MUL = mybir.AluOpType.mult
GT = mybir.AluOpType.is_gt
X = mybir.AxisListType.X
EXP = mybir.ActivationFunctionType.Exp

N_CORES = 8
B, K, L = 8192, 4096, 6
NS = [32, 16, 8, 4, 2, 1]            # streams per layer
FO = np.cumsum([0] + NS).tolist()    # stream offsets by layer
COLB = [f * 4 for f in FO]           # column base per layer
BROW = B // 128

GSPLIT = True                        # GpSimd evaluates pattern 3

_compiled = None


def _build_program():
    nc = bacc.Bacc("TRN2", target_bir_lowering=False, debug=False,
                   num_devices=N_CORES)
    wallin = nc.dram_tensor("wallin", [128, 4096], FP8, kind="ExternalInput").ap()
    abin = nc.dram_tensor("abin", [128, 1024], BF16, kind="ExternalInput").ap()
    xkin = nc.dram_tensor("xkin", [128, 168], BF16, kind="ExternalInput").ap()
    out = nc.dram_tensor("out", [128, BROW], F32, kind="ExternalOutput").ap()

    with tile.TileContext(nc) as tc:
        with ExitStack() as ctx:
            p = ctx.enter_context(tc.tile_pool(name="p", bufs=1))
            psp = ctx.enter_context(tc.tile_pool(name="ps", bufs=1, space="PSUM"))

            # ---- input DMAs first: completion latency overlaps setup ----
            # xk (x rows + coeff matrix) rides the scalar HWDGE queue first
            # so kmv is resident before the first matmul; wall chunks are
            # interleaved sync/scalar so each exp chunk lands just in time.
            wall = p.tile([128, 4096], FP8)
            xk = p.tile([128, 168], BF16)
            ab = p.tile([128, 1024], BF16)
            nc.sync.dma_start(wall[:, 0:1024], wallin[:, 0:1024])
            nc.scalar.dma_start(xk[:], xkin[:])
            nc.sync.dma_start(wall[:, 2048:4096], wallin[:, 2048:4096])
            nc.scalar.dma_start(wall[:, 1024:2048], wallin[:, 1024:2048])
            nc.gpsimd.dma_start(ab[:], abin[:])

            av = ab[:, 0:512].rearrange("p (q c) -> p q c", c=128)
            bv = ab[:, 512:1024].rearrange("p (q c) -> p q c", c=128)
            xv = xk[:, 0:128].rearrange("p (a c) -> p a c", c=2)
            kmv = xk[:, 128:168]

            # ones for the table-broadcast matmul, built on device
            onesb = p.tile([128, 128], BF16)
            nc.gpsimd.memset(onesb[:], 1.0)

            # blend prep on DVE while weights are in flight
            s0 = p.tile([128, BROW], F32)
            nc.vector.tensor_scalar(s0[:], xv[:, :, 0], 0.0, None, op0=GT)
            s1 = p.tile([128, BROW], F32)
            nc.vector.tensor_scalar(s1[:], xv[:, :, 1], 0.0, None, op0=GT)
            t01 = p.tile([128, BROW], F32)
            nc.vector.tensor_tensor(t01[:], s0[:], s1[:], op=MUL)

            # ---- exp on ACT, 4 chunks ----
            E = p.tile([128, 4096], BF16)
            for k in range(4):
                nc.scalar.activation(E[:, k * 1024:(k + 1) * 1024],
                                     wall[:, k * 1024:(k + 1) * 1024], EXP)

            # ---- coefficient matmuls: one per 8-column chunk ----
            psb = [psp.tile([128, 320], F32, tag=f"pb{b}", name=f"pb{b}")
                   for b in range(4)]
            for c in range(32):
                b, s = c // 8, c % 8
                nc.tensor.matmul(psb[b][:, s * 40:(s + 1) * 40],
                                 E[:, c * 128:(c + 1) * 128], kmv,
                                 start=True, stop=True)

            # ---- per-bank: r = 1/D, then normalize+convert coefficients ----
            slabK = p.tile([128, 4, 256], BF16)
            rall = p.tile([128, 256], F32)

            def recip_bank(b):
                dflat = psb[b][:].rearrange("p (sg k) -> p sg k", k=5)
                rv = rall[:, b * 64:(b + 1) * 64]
                nc.vector.reciprocal_approx_fast(rv, dflat[:, :, 4])

            def norm_bank(b, eng):
                pv = psb[b][:].rearrange("p (s g k) -> p k s g", k=5, g=8)
                outv = slabK[:, :, b * 64:(b + 1) * 64].rearrange(
                    "p k (s g) -> p k s g", g=8)
                rb = rall[:, b * 64:(b + 1) * 64].rearrange(
                    "p (s g) -> p s g", g=8).unsqueeze(1).broadcast_to(
                    [128, 4, 8, 8])
                eng.tensor_tensor(outv, pv[:, 0:4], rb, op=MUL)

            H = {}
            for l in range(L):
                n = COLB[l + 1] - COLB[l]
                H[l] = p.tile([128, 4, n], BF16, tag=f"H{l}", name=f"H{l}")

            def eval_piece(l, lo, hi, tag, eng, q0, q1):
                # V = [t | u] double-wide: t = B*c3 + c1, u = B*c2 + c0,
                # then H = t*A + u.  Patterns [q0:q1) only.
                n = hi - lo
                nq = q1 - q0
                if l == 0:
                    A = av[:, q0:q1, lo:hi]
                    BB = bv[:, q0:q1, lo:hi].unsqueeze(2).broadcast_to(
                        [128, nq, 2, n])
                else:
                    Hp = H[l - 1]
                    A = Hp[:, q0:q1, 0:n]
                    BB = Hp[:, q0:q1, n:2 * n].unsqueeze(2).broadcast_to(
                        [128, nq, 2, n])
                c32 = slabK[:, 0:2, lo:hi].unsqueeze(1).broadcast_to(
                    [128, nq, 2, n])
                c10 = slabK[:, 2:4, lo:hi].unsqueeze(1).broadcast_to(
                    [128, nq, 2, n])
                llo = lo - COLB[l]
                Hv = H[l][:, q0:q1, llo:llo + n]
                V = p.tile([128, nq, 2, n], BF16, tag=f"V{tag}", name=f"V{tag}")
                eng.tensor_tensor(V[:], BB, c32, op=MUL)
                eng.tensor_tensor(V[:], V[:], c10, op=ADD)
                m = p.tile([128, nq, n], BF16, tag=f"m{tag}", name=f"m{tag}")
                eng.tensor_tensor(m[:], V[:, :, 0], A, op=MUL)
                eng.tensor_tensor(Hv, m[:], V[:, :, 1], op=ADD)

            PIECES = [(0, 0, 64, "l0a"), (0, 64, 128, "l0b"),
                      (1, 128, 192, "l1"), (2, 192, 224, "l2"),
                      (3, 224, 240, "l3"), (4, 240, 248, "l4"),
                      (5, 248, 252, "l5")]

            def vpiece(i):
                l, lo, hi, tag = PIECES[i]
                eval_piece(l, lo, hi, tag + "v", nc.vector, 0, 4)

            # DVE runs recips + the serial eval chain; GpSimd runs the
            # PSUM->slab norms for banks 1-3 in parallel (bank 0 stays on
            # DVE so the chain starts without a cross-engine hop).
            recip_bank(0)
            norm_bank(0, nc.vector)
            vpiece(0)
            recip_bank(1)
            norm_bank(1, nc.vector)
            vpiece(1)
            recip_bank(2)
            norm_bank(2, nc.vector)
            vpiece(2)
            recip_bank(3)
            norm_bank(3, nc.vector)
            for i in range(3, 7):
                vpiece(i)

            # ---- partial GroupSum: ones-matmul sums H5 over partitions ----
            psg = psp.tile([128, 16], F32)
            nc.tensor.matmul(psg[:], onesb[:],
                             H[5][:].rearrange("p q c -> p (q c)"),
                             start=True, stop=True)

            pr = psg[:].rearrange("p (q c) -> p q c", c=4)
            T = p.tile([128, 4], F32)
            nc.vector.tensor_reduce(T[:], pr[:], axis=X, op=ADD)
            # U = [d10, d01, T11-T00];  d11 = U2 - U0 - U1
            U = p.tile([128, 3], F32)
            nc.vector.tensor_tensor(U[:], T[:, 1:4],
                                    T[:, 0:1].broadcast_to([128, 3]), op=SUB)
            dt = p.tile([128, 1], F32)
            nc.vector.scalar_tensor_tensor(dt[:], U[:, 2:3], U[:, 0:1],
                                           U[:, 1:2], op0=SUB, op1=SUB)

            # ---- per-row blend of the full batch (scalars from tail tiles) ----
            ev = p.tile([128, BROW], F32)
            nc.vector.tensor_scalar(ev[:], s0[:], U[:, 0:1], T[:, 0:1],
                                    op0=MUL, op1=ADD)
            z1 = p.tile([128, BROW], F32)
            nc.vector.scalar_tensor_tensor(z1[:], s1[:], U[:, 1:2], ev[:],
                                           op0=MUL, op1=ADD)
            zo = p.tile([128, BROW], F32)
            nc.vector.scalar_tensor_tensor(zo[:], t01[:], dt[:, 0:1], z1[:],
                                           op0=MUL, op1=ADD)
            nc.sync.dma_start(out, zo[:])

    nc.compile()
    return nc


def _host_blobs(x, w0, ws, idx0, idxs):
    """Compose the stream tree and build per-core input blobs."""
    x = np.asarray(x, np.float32)
    Wl = [np.asarray(w0, np.float32)] + [np.asarray(ws[i], np.float32)
                                         for i in range(L - 1)]
    Il = [np.asarray(idx0, np.int64)] + [np.asarray(idxs[i], np.int64)
                                         for i in range(L - 1)]

    S = [None] * L
    S[L - 1] = [np.arange(K)]
    for l in range(L - 1, 0, -1):
        S[l - 1] = [Il[l][0][P] for P in S[l]] + [Il[l][1][P] for P in S[l]]

    # weights in column order: wtmp[core, col, p, i], col = 4*stream + j
    wtmp = np.zeros((N_CORES, 256, 128, 16), np.float32)
    for l in range(L):
        for s in range(NS[l]):
            sg = FO[l] + s
            pw = Wl[l][S[l][s]].reshape(N_CORES, 4, 128, 16)
            wtmp[:, sg * 4:(sg + 1) * 4] = pw
    # wall[core, g*16+i, c*128+p], col = c*8+g
    wt = wtmp.reshape(N_CORES, 32, 8, 128, 16)
    wall = np.ascontiguousarray(
        wt.transpose(0, 2, 4, 1, 3).reshape(N_CORES, 128, 4096))

    # layer-0 pattern inputs, pattern-major: a0[core, p, q*128 + col]
    q = np.arange(4)
    msel0 = np.zeros((N_CORES, 128, 128), np.int64)  # [core, col, p]
    msel1 = np.zeros((N_CORES, 128, 128), np.int64)
    for s in range(NS[0]):
        idx = S[0][s].reshape(N_CORES, 4, 128)
        msel0[:, s * 4:(s + 1) * 4] = Il[0][0][idx]
        msel1[:, s * 4:(s + 1) * 4] = Il[0][1][idx]
    a0 = (q[None, :, None, None] >> msel0[:, None, :, :]) & 1   # [core,q,col,p]
    b0 = (q[None, :, None, None] >> msel1[:, None, :, :]) & 1
    a0 = a0.transpose(0, 3, 1, 2).reshape(N_CORES, 128, 512)    # [core,p,(q,col)]
    b0 = b0.transpose(0, 3, 1, 2).reshape(N_CORES, 128, 512)

    # constant 16->5 coefficient matrix, block-diagonal over 8 groups
    i16 = np.arange(16)
    t11, t10 = i16 & 1, (i16 >> 1) & 1
    t01, t00 = (i16 >> 2) & 1, (i16 >> 3) & 1
    KC = np.stack([t11 - t10 - t01 + t00, t01 - t00, t10 - t00,
                   t00, np.ones(16, np.int64)], 1)  # [16,5]: c3,c2,c1,c0,D
    kb = np.zeros((8, 16, 8, 5), np.float32)
    for gidx in range(8):
        kb[gidx, :, gidx, :] = KC
    kblob = kb.reshape(128, 40)

    xpart = np.ascontiguousarray(x.reshape(128, 128))

    xkm = np.concatenate([xpart, kblob], axis=1)
    in_maps = []
    for ci in range(N_CORES):
        abm = np.concatenate([a0[ci], b0[ci]], axis=1)
        in_maps.append({
            "wallin": wall[ci].astype(ml_dtypes.float8_e4m3fn),
            "abin": np.ascontiguousarray(abm).astype(ml_dtypes.bfloat16),
            "xkin": np.ascontiguousarray(xkm).astype(ml_dtypes.bfloat16),
        })
    return in_maps


def run(inputs, trace=False, trace_kwargs=None):
    global _compiled
    if _compiled is None:
        _compiled = _build_program()
    nc = _compiled
    in_maps = _host_blobs(inputs["x"], inputs["w0"], inputs["ws"],
                          inputs["idx0"], inputs["idxs"])
    res = run_bass_kernel_spmd(nc, in_maps, core_ids=list(range(N_CORES)),
                               trace=trace, **(trace_kwargs or {}))
    total = np.zeros((B, 2), np.float32)
    for ci in range(N_CORES):
        total[:, 0 if ci < N_CORES // 2 else 1] += \
            np.asarray(res.results[ci]["out"], np.float32).reshape(B)
    return total, res


def kernel(x, w0, ws, idx0, idxs):
    out, _ = run({"x": x, "w0": w0, "ws": ws, "idx0": idx0, "idxs": idxs})
    return out


# revision 11
# speedup vs baseline: 1.1063x; 1.0231x over previous
"""Trainium2 Bass kernel for nn_DiffLogicPBF (difflogic network).

Algorithm
---------
The network input is binarized to 2 bits, so every batch row's activation
trajectory takes one of only 4 "patterns".  We evaluate the network on the 4
patterns instead of 8192 rows, then blend per-row.

The per-layer gathers are composed on the host into a stream tree (layer l is
evaluated 2^(5-l) times in permuted orders, 63 streams total), so the device
does gather-free elementwise work only.  Weights arrive pre-permuted, in fp8.

Device pipeline per core (512 neurons x 63 streams = 252 columns of 128):
  exp(w) on ACT  ->  PE matmul with a constant 16->5 matrix that computes the
  multilinear coefficients c0..c3 and the softmax denominator D per neuron
  ->  r = 1/D per column, then one fused PSUM->SBUF multiply that both
  converts and NORMALIZES the coefficients (slab = psum * r)  ->
  elementwise multilinear evaluation over the 4 patterns; the pattern dim is
  split across engines (DVE takes patterns 0-2, GpSimd pattern 3) so the two
  engines evaluate the serial layer chain in parallel.

Tail: the 4-entry logit table is summed over partitions by a ones-matmul on
H5 directly, folded, differenced, and blended per batch row with 3 fused ops.
Each core emits one [128,64] plane (its class partial); the host assembles.

Sharding: neurons are split across the 8 cores (512 each); cores 0-3 are
class-0 partials, 4-7 class-1.
"""

from contextlib import ExitStack

import ml_dtypes
import numpy as np

import concourse.bacc as bacc
import concourse.mybir as mybir
import concourse.tile as tile
from concourse.bass_utils import run_bass_kernel_spmd

F32 = mybir.dt.float32
BF16 = mybir.dt.bfloat16
FP8 = mybir.dt.float8e4
ADD = mybir.AluOpType.add
SUB = mybir.AluOpType.subtract
MUL = mybir.AluOpType.mult
GT = mybir.AluOpType.is_gt
X = mybir.AxisListType.X
EXP = mybir.ActivationFunctionType.Exp

N_CORES = 8
B, K, L = 8192, 4096, 6
NS = [32, 16, 8, 4, 2, 1]            # streams per layer
FO = np.cumsum([0] + NS).tolist()    # stream offsets by layer
COLB = [f * 4 for f in FO]           # column base per layer
BROW = B // 128

GSPLIT = True                        # GpSimd evaluates pattern 3

_compiled = None


def _build_program():
    nc = bacc.Bacc("TRN2", target_bir_lowering=False, debug=False,
                   num_devices=N_CORES)
    wallin = nc.dram_tensor("wallin", [128, 4096], FP8, kind="ExternalInput").ap()
    abin = nc.dram_tensor("abin", [128, 1024], BF16, kind="ExternalInput").ap()
    xkin = nc.dram_tensor("xkin", [128, 168], BF16, kind="ExternalInput").ap()
    out = nc.dram_tensor("out", [128, BROW], F32, kind="ExternalOutput").ap()

    with tile.TileContext(nc) as tc:
        with ExitStack() as ctx:
            p = ctx.enter_context(tc.tile_pool(name="p", bufs=1))
            psp = ctx.enter_context(tc.tile_pool(name="ps", bufs=1, space="PSUM"))

            # ---- input DMAs first: completion latency overlaps setup ----
            # xk (x rows + coeff matrix) rides the scalar HWDGE queue first
            # so kmv is resident before the first matmul; wall chunks are
            # interleaved sync/scalar so each exp chunk lands just in time.
            wall = p.tile([128, 4096], FP8)
            xk = p.tile([128, 168], BF16)
            ab = p.tile([128, 1024], BF16)
            nc.sync.dma_start(wall[:, 0:1024], wallin[:, 0:1024])
            nc.scalar.dma_start(xk[:], xkin[:])
            nc.gpsimd.dma_start(wall[:, 1024:2048], wallin[:, 1024:2048])
            nc.sync.dma_start(wall[:, 2048:3072], wallin[:, 2048:3072])
            nc.scalar.dma_start(wall[:, 3072:4096], wallin[:, 3072:4096])
            nc.gpsimd.dma_start(ab[:], abin[:])

            av = ab[:, 0:512].rearrange("p (q c) -> p q c", c=128)
            bv = ab[:, 512:1024].rearrange("p (q c) -> p q c", c=128)
            xv = xk[:, 0:128].rearrange("p (a c) -> p a c", c=2)
            kmv = xk[:, 128:168]

            # ones for the table-broadcast matmul, built on device
            onesb = p.tile([128, 128], BF16)
            nc.gpsimd.memset(onesb[:], 1.0)

            # blend prep on DVE while weights are in flight
            s0 = p.tile([128, BROW], F32)
            nc.vector.tensor_scalar(s0[:], xv[:, :, 0], 0.0, None, op0=GT)
            s1 = p.tile([128, BROW], F32)
            nc.vector.tensor_scalar(s1[:], xv[:, :, 1], 0.0, None, op0=GT)
            t01 = p.tile([128, BROW], F32)
            nc.vector.tensor_tensor(t01[:], s0[:], s1[:], op=MUL)

            # ---- exp on ACT, 4 chunks ----
            E = p.tile([128, 4096], BF16)
            for k in range(4):
                nc.scalar.activation(E[:, k * 1024:(k + 1) * 1024],
                                     wall[:, k * 1024:(k + 1) * 1024], EXP)

            # ---- coefficient matmuls: one per 8-column chunk ----
            psb = [psp.tile([128, 320], F32, tag=f"pb{b}", name=f"pb{b}")
                   for b in range(4)]
            for c in range(32):
                b, s = c // 8, c % 8
                nc.tensor.matmul(psb[b][:, s * 40:(s + 1) * 40],
                                 E[:, c * 128:(c + 1) * 128], kmv,
                                 start=True, stop=True)

            # ---- per-bank: r = 1/D, then normalize+convert coefficients ----
            slabK = p.tile([128, 4, 256], BF16)
            rall = p.tile([128, 256], F32)

            def recip_bank(b):
                dflat = psb[b][:].rearrange("p (sg k) -> p sg k", k=5)
                rv = rall[:, b * 64:(b + 1) * 64]
                nc.vector.reciprocal_approx_fast(rv, dflat[:, :, 4])

            def norm_bank(b, eng):
                pv = psb[b][:].rearrange("p (s g k) -> p k s g", k=5, g=8)
                outv = slabK[:, :, b * 64:(b + 1) * 64].rearrange(
                    "p k (s g) -> p k s g", g=8)
                rb = rall[:, b * 64:(b + 1) * 64].rearrange(
                    "p (s g) -> p s g", g=8).unsqueeze(1).broadcast_to(
                    [128, 4, 8, 8])
                eng.tensor_tensor(outv, pv[:, 0:4], rb, op=MUL)

            H = {}
            for l in range(L):
                n = COLB[l + 1] - COLB[l]
                H[l] = p.tile([128, 4, n], BF16, tag=f"H{l}", name=f"H{l}")

            def eval_piece(l, lo, hi, tag, eng, q0, q1):
                # V = [t | u] double-wide: t = B*c3 + c1, u = B*c2 + c0,
                # then H = t*A + u.  Patterns [q0:q1) only.
                n = hi - lo
                nq = q1 - q0
                if l == 0:
                    A = av[:, q0:q1, lo:hi]
                    BB = bv[:, q0:q1, lo:hi].unsqueeze(2).broadcast_to(
                        [128, nq, 2, n])
                else:
                    Hp = H[l - 1]
                    A = Hp[:, q0:q1, 0:n]
                    BB = Hp[:, q0:q1, n:2 * n].unsqueeze(2).broadcast_to(
                        [128, nq, 2, n])
                c32 = slabK[:, 0:2, lo:hi].unsqueeze(1).broadcast_to(
                    [128, nq, 2, n])
                c10 = slabK[:, 2:4, lo:hi].unsqueeze(1).broadcast_to(
                    [128, nq, 2, n])
                llo = lo - COLB[l]
                Hv = H[l][:, q0:q1, llo:llo + n]
                V = p.tile([128, nq, 2, n], BF16, tag=f"V{tag}", name=f"V{tag}")
                eng.tensor_tensor(V[:], BB, c32, op=MUL)
                eng.tensor_tensor(V[:], V[:], c10, op=ADD)
                m = p.tile([128, nq, n], BF16, tag=f"m{tag}", name=f"m{tag}")
                eng.tensor_tensor(m[:], V[:, :, 0], A, op=MUL)
                eng.tensor_tensor(Hv, m[:], V[:, :, 1], op=ADD)

            PIECES = [(0, 0, 64, "l0a"), (0, 64, 128, "l0b"),
                      (1, 128, 192, "l1"), (2, 192, 224, "l2"),
                      (3, 224, 240, "l3"), (4, 240, 248, "l4"),
                      (5, 248, 252, "l5")]

            def vpiece(i):
                l, lo, hi, tag = PIECES[i]
                eval_piece(l, lo, hi, tag + "v", nc.vector, 0, 4)

            # DVE runs recips + the serial eval chain; GpSimd runs the
            # PSUM->slab norms for banks 1-3 in parallel (bank 0 stays on
            # DVE so the chain starts without a cross-engine hop).
            # wait floors keep the scheduler from hoisting later banks'
            # PSUM-gated ops ahead of ready eval ops (DVE head-of-line)
            recip_bank(0)
            norm_bank(0, nc.vector)
            vpiece(0)
            with tc.tile_wait_until(0.008):
                recip_bank(1)
                norm_bank(1, nc.vector)
            vpiece(1)
            with tc.tile_wait_until(0.0095):
                recip_bank(2)
                norm_bank(2, nc.vector)
            vpiece(2)
            with tc.tile_wait_until(0.011):
                recip_bank(3)
                norm_bank(3, nc.vector)
            for i in range(3, 7):
                vpiece(i)

            # ---- partial GroupSum: ones-matmul sums H5 over partitions ----
            psg = psp.tile([128, 16], F32)
            nc.tensor.matmul(psg[:], onesb[:],
                             H[5][:].rearrange("p q c -> p (q c)"),
                             start=True, stop=True)

            pr = psg[:].rearrange("p (q c) -> p q c", c=4)
            T = p.tile([128, 4], F32)
            nc.vector.tensor_reduce(T[:], pr[:], axis=X, op=ADD)
            # U = [d10, d01, T11-T00];  d11 = U2 - U0 - U1
            U = p.tile([128, 3], F32)
            nc.vector.tensor_tensor(U[:], T[:, 1:4],
                                    T[:, 0:1].broadcast_to([128, 3]), op=SUB)
            dt = p.tile([128, 1], F32)
            nc.vector.scalar_tensor_tensor(dt[:], U[:, 2:3], U[:, 0:1],
                                           U[:, 1:2], op0=SUB, op1=SUB)

            # ---- per-row blend of the full batch (scalars from tail tiles) ----
            ev = p.tile([128, BROW], F32)
            nc.vector.tensor_scalar(ev[:], s0[:], U[:, 0:1], T[:, 0:1],
                                    op0=MUL, op1=ADD)
            z1 = p.tile([128, BROW], F32)
            nc.vector.scalar_tensor_tensor(z1[:], s1[:], U[:, 1:2], ev[:],
                                           op0=MUL, op1=ADD)
            zo = p.tile([128, BROW], F32)
            nc.vector.scalar_tensor_tensor(zo[:], t01[:], dt[:, 0:1], z1[:],
                                           op0=MUL, op1=ADD)
            nc.sync.dma_start(out, zo[:])

    nc.compile()
    return nc


def _host_blobs(x, w0, ws, idx0, idxs):
    """Compose the stream tree and build per-core input blobs."""
    x = np.asarray(x, np.float32)
    Wl = [np.asarray(w0, np.float32)] + [np.asarray(ws[i], np.float32)
                                         for i in range(L - 1)]
    Il = [np.asarray(idx0, np.int64)] + [np.asarray(idxs[i], np.int64)
                                         for i in range(L - 1)]

    S = [None] * L
    S[L - 1] = [np.arange(K)]
    for l in range(L - 1, 0, -1):
        S[l - 1] = [Il[l][0][P] for P in S[l]] + [Il[l][1][P] for P in S[l]]

    # weights in column order: wtmp[core, col, p, i], col = 4*stream + j
    wtmp = np.zeros((N_CORES, 256, 128, 16), np.float32)
    for l in range(L):
        for s in range(NS[l]):
            sg = FO[l] + s
            pw = Wl[l][S[l][s]].reshape(N_CORES, 4, 128, 16)
            wtmp[:, sg * 4:(sg + 1) * 4] = pw
    # wall[core, g*16+i, c*128+p], col = c*8+g
    wt = wtmp.reshape(N_CORES, 32, 8, 128, 16)
    wall = np.ascontiguousarray(
        wt.transpose(0, 2, 4, 1, 3).reshape(N_CORES, 128, 4096))

    # layer-0 pattern inputs, pattern-major: a0[core, p, q*128 + col]
    q = np.arange(4)
    msel0 = np.zeros((N_CORES, 128, 128), np.int64)  # [core, col, p]
    msel1 = np.zeros((N_CORES, 128, 128), np.int64)
    for s in range(NS[0]):
        idx = S[0][s].reshape(N_CORES, 4, 128)
        msel0[:, s * 4:(s + 1) * 4] = Il[0][0][idx]
        msel1[:, s * 4:(s + 1) * 4] = Il[0][1][idx]
    a0 = (q[None, :, None, None] >> msel0[:, None, :, :]) & 1   # [core,q,col,p]
    b0 = (q[None, :, None, None] >> msel1[:, None, :, :]) & 1
    a0 = a0.transpose(0, 3, 1, 2).reshape(N_CORES, 128, 512)    # [core,p,(q,col)]
    b0 = b0.transpose(0, 3, 1, 2).reshape(N_CORES, 128, 512)

    # constant 16->5 coefficient matrix, block-diagonal over 8 groups
    i16 = np.arange(16)
    t11, t10 = i16 & 1, (i16 >> 1) & 1
    t01, t00 = (i16 >> 2) & 1, (i16 >> 3) & 1
    KC = np.stack([t11 - t10 - t01 + t00, t01 - t00, t10 - t00,
                   t00, np.ones(16, np.int64)], 1)  # [16,5]: c3,c2,c1,c0,D
    kb = np.zeros((8, 16, 8, 5), np.float32)
    for gidx in range(8):
        kb[gidx, :, gidx, :] = KC
    kblob = kb.reshape(128, 40)

    xpart = np.ascontiguousarray(x.reshape(128, 128))

    xkm = np.concatenate([xpart, kblob], axis=1)
    in_maps = []
    for ci in range(N_CORES):
        abm = np.concatenate([a0[ci], b0[ci]], axis=1)
        in_maps.append({
            "wallin": wall[ci].astype(ml_dtypes.float8_e4m3fn),
            "abin": np.ascontiguousarray(abm).astype(ml_dtypes.bfloat16),
            "xkin": np.ascontiguousarray(xkm).astype(ml_dtypes.bfloat16),
        })
    return in_maps


def run(inputs, trace=False, trace_kwargs=None):
    global _compiled
    if _compiled is None:
        _compiled = _build_program()
    nc = _compiled
    in_maps = _host_blobs(inputs["x"], inputs["w0"], inputs["ws"],
                          inputs["idx0"], inputs["idxs"])
    res = run_bass_kernel_spmd(nc, in_maps, core_ids=list(range(N_CORES)),
                               trace=trace, **(trace_kwargs or {}))
    total = np.zeros((B, 2), np.float32)
    for ci in range(N_CORES):
        total[:, 0 if ci < N_CORES // 2 else 1] += \
            np.asarray(res.results[ci]["out"], np.float32).reshape(B)
    return total, res


def kernel(x, w0, ws, idx0, idxs):
    out, _ = run({"x": x, "w0": w0, "ws": ws, "idx0": idx0, "idxs": idxs})
    return out
